# revision 43
# baseline (speedup 1.0000x reference)
"""BagOfWordsMLP on 8 Trainium2 NeuronCores.

Strategy (vocab-sharded fc1 + single ReduceScatter, transposed layout):
  h1 = relu(bow @ W1 + b1) over a [B=1024, V=50257] histogram. Each core
  owns a 6283-row vocab shard of W1 (pre-scaled by S1, fp8e4m3) and a
  dense fp8 count matrix [6400 slots, 1024 rows] built host-side during
  input sharding. fc1 runs TRANSPOSED on the PE: out = [H1-part,
  batch-free] via DoubleRow fp8 matmuls (w1 stationary, counts moving),
  so partials reach fc2/fc3 already transposed and the tail needs no PE
  transposes or identity matrix. b1 is folded in as an extra vocab slot
  (row = b1*S1/8, count 1).

  PSUM holds 8 of the 16 accumulator groups (8 H1-chunks x 2
  batch-halves). The halves ping-pong through PSUM in the paired round
  order 0A 1A 1B 0B 0C 1C 1D 0D (only 3 occupant changes), with partial
  sums parked in f32 SBUF checkpoints. Occupant changes never reload:
  the incoming half starts fresh (start=True) and its checkpoint is
  merged back by tensor-add at the next park/drain, so transition
  copies are halved and the PE never stalls on a checkpoint round-trip.

  Partials are summed across cores with ONE ReduceScatter of the full
  [B, H1] partial (the cost model charges a 15us constant per
  collective, so one big RS beats two half-RS by 15us). Partials DRAM
  rows are (blk, p, hc) so every DMA descriptor is >= 1KB contiguous,
  and the drain DMAs go out in hc-halves/quarters to overlap the
  final matmuls.
  Each core then computes relu, fc2, fc3 for its own 128 batch rows in
  bf16 with h2 also produced transposed (biases folded in as K=1
  matmul rows).
"""

import os
import sys

import numpy as np

sys.path.insert(0, "/opt/trn_rl_repo")
os.environ.setdefault("JAX_PLATFORMS", "axon,cpu")

import ml_dtypes  # noqa: E402

from concourse import bacc, bass, mybir, tile  # noqa: E402,F401
from concourse.bass_utils import run_bass_kernel_spmd  # noqa: E402

BF16 = ml_dtypes.bfloat16
F8E4 = ml_dtypes.float8_e4m3

N_CORES = 8
B, S = 1024, 512
B_LOC = B // N_CORES  # 128 rows per core
V = 50257
H1, H2, C = 1024, 512, 20

SH = -(-V // N_CORES)  # 6283 vocab rows per shard (last shard 6276)
VSH = 6400  # padded shard slots: 50 k-subtiles, 25 DoubleRow chunks
KSUB = VSH // 128  # 50
NKC = VSH // 256  # 25 DoubleRow chunks
NHC = H1 // 128  # 8 H1 chunks
S1 = 32768.0  # fp8 dequant scale for W1 (max |W1*S1| ~ 146 < e4m3 max)
DR = mybir.MatmulPerfMode.DoubleRow

# kc segments A-D for the PSUM ping-pong (see module docstring)
SEGA, SEGB, SEGC, SEGD = (0, 5), (5, 11), (11, 19), (19, NKC)

# wpk layout: w2T blocks (kc-outer, hc2-inner) | woutT blocks
W2T_COLS = NHC * 4 * 128  # 4096
WOUT_OFF = W2T_COLS
WPK_COLS = WOUT_OFF + 4 * C
# consts layout: ones(128) | bout(C) | b2(512)
B2_OFF = 128 + C
CST_COLS = B2_OFF + H2

LAST_EXEC_NS = None
_NC_CACHE = None


def _build_program():
    nc = bacc.Bacc(
        "TRN2", target_bir_lowering=False, debug=False, num_devices=N_CORES
    )
    f32 = mybir.dt.float32
    bf16 = mybir.dt.bfloat16
    f8e4 = mybir.dt.float8e4
    add = mybir.AluOpType.add

    w1s = nc.declare_dram_parameter("w1s", [128, KSUB, H1], f8e4, isOutput=False)
    cnts = nc.declare_dram_parameter("cnts", [128, KSUB, B], f8e4, isOutput=False)
    wpk = nc.declare_dram_parameter("wpk", [128, WPK_COLS], bf16, isOutput=False)
    consts = nc.declare_dram_parameter("consts", [1, CST_COLS], bf16, isOutput=False)
    out_d = nc.declare_dram_parameter("out", [B_LOC, C], f32, isOutput=True)

    with tile.TileContext(nc) as tc:
        with (
            tc.tile_pool(name="wpool", bufs=1) as wpool,
            tc.tile_pool(name="hpool", bufs=1) as hpool,
            tc.tile_pool(name="ppool", bufs=1, space="PSUM") as ppool,
            tc.tile_pool(name="dram", bufs=1, space="DRAM") as dram,
        ):
            # partials rows = (blk, p, hc); RS scatters blk-major so core k
            # receives batch block k as h1^T [(p hc), bw] with 2KB-contiguous
            # per-partition runs.
            partials = dram.tile([8 * 128 * NHC, 128], bf16, tag="partials",
                                 name="partials")
            rs_out = dram.tile([128 * NHC, 128], bf16, tag="rs_out",
                               name="rs_out")

            # --- PE warmup: junk matmuls on bank0 while the first stream
            # chunk is in flight, so the pstate ramp completes early ---
            zl = wpool.tile([1, 128], bf16)
            zr = wpool.tile([1, 512], bf16)
            nc.vector.memset(zl[:], 0.0)
            nc.vector.memset(zr[:], 0.0)

            # --- stream W1 shard + counts into SBUF (w1 on the SP HWDGE
            # queue, counts on the Pool SWDGE queue) ---
            cst = wpool.tile([1, CST_COLS], bf16)
            nc.scalar.dma_start(out=cst[:], in_=consts[:])
            wpk_sb = wpool.tile([128, WPK_COLS], bf16)
            w1_sb = wpool.tile([128, KSUB, H1], f8e4)
            cnt_sb = wpool.tile([128, KSUB, B], f8e4)
            bounds = [0, 2] + list(range(6, KSUB + 1, 4))
            for i in range(len(bounds) - 1):
                k0, k1 = bounds[i], bounds[i + 1]
                nc.sync.dma_start(out=w1_sb[:, k0:k1, :], in_=w1s[:, k0:k1, :])
                nc.gpsimd.dma_start(out=cnt_sb[:, k0:k1, :], in_=cnts[:, k0:k1, :])
            # fc2/fc3 weights ride the SP queue behind the last w1 chunk
            nc.sync.dma_start(out=wpk_sb[:], in_=wpk[:])

            # stage free layout (bh, blk, hc, bw): per-partition (hc, bw)
            # contiguous within blk -> >=1KB DMA descriptors
            stage = hpool.tile([128, 2, 4, NHC, 128], bf16, tag="stage",
                               name="stage")
            msts = {
                (bh, hc): hpool.tile(
                    [128, 512], f32, tag=f"mst{bh}_{hc}", name=f"mst{bh}_{hc}"
                )
                for bh in range(2)
                for hc in range(NHC)
            }
            tmps = [
                hpool.tile([128, 512], f32, tag=f"tmp{hc}", name=f"tmp{hc}")
                for hc in range(NHC)
            ]

            banks = [
                ppool.tile([128, 512], f32, tag=f"bank{hc}", name=f"bank{hc}")
                for hc in range(NHC)
            ]

            for j in range(4):
                nc.tensor.matmul(
                    banks[0][:], zl[:], zr[:],
                    start=(j == 0), stop=(j == 3), skip_group_check=True,
                )

            def mm(bh, hc, kc, start, stop=False):
                nc.tensor.matmul(
                    banks[hc][:],
                    w1_sb[:, 2 * kc : 2 * kc + 2, hc * 128 : (hc + 1) * 128],
                    cnt_sb[:, 2 * kc : 2 * kc + 2, bh * 512 : (bh + 1) * 512],
                    start=start,
                    stop=stop,
                    perf_mode=DR,
                    skip_group_check=True,
                )

            def kc_inner(bh, seg, fresh):
                a, b = seg
                for kc in range(a, b):
                    for hc in range(NHC):
                        mm(bh, hc, kc, start=(fresh and kc == a),
                           stop=(kc == b - 1))

            def ckpt(bh, hc, eng):
                # park the current psum into this half's checkpoint
                if eng == 0:
                    nc.scalar.activation(
                        msts[(bh, hc)][:], banks[hc][:],
                        mybir.ActivationFunctionType.Copy,
                    )
                else:
                    nc.vector.tensor_copy(msts[(bh, hc)][:], banks[hc][:])

            # R0: bh0 segment A, paced by the stream
            kc_inner(0, SEGA, fresh=True)
            # R1: bh1 segment A; per-bank: park bh0 (alternating Act/DVE),
            # then this bank's matmuls (fresh)
            for hc in range(NHC):
                ckpt(0, hc, 1 - hc % 2)
                for kc in range(*SEGA):
                    mm(1, hc, kc, start=(kc == SEGA[0]))
            # R2: bh1 segment B (continue)
            kc_inner(1, SEGB, fresh=False)
            # R3: bh0 segment B; per-bank: park bh1, matmuls fresh
            for hc in range(NHC):
                ckpt(1, hc, 1 - hc % 2)
                for kc in range(*SEGB):
                    mm(0, hc, kc, start=(kc == SEGB[0]))
            # R4: bh0 segment C (continue)
            kc_inner(0, SEGC, fresh=False)
            # R5: bh1 segment C; per-bank: merge bh0's B+C into its
            # checkpoint (DVE add, psum-capable), matmuls fresh
            for hc in range(NHC):
                nc.vector.tensor_add(
                    msts[(0, hc)][:], banks[hc][:], msts[(0, hc)][:]
                )
                for kc in range(*SEGC):
                    mm(1, hc, kc, start=(kc == SEGC[0]))
            def drain_dma(bh, h0, n, alt=False):
                ap = partials[bh * 4096 : (bh + 1) * 4096, :].rearrange(
                    "(blk p hc) bw -> p blk hc bw", blk=4, p=128
                )
                # bh1 ships on the Act queue, bh0 alternates SP/Act so no
                # quarter queues behind the previous one's issue
                eng = nc.scalar if (bh == 1 or alt) else nc.sync
                eng.dma_start(
                    out=ap[:, :, h0 : h0 + n, :],
                    in_=stage[:, bh, :, h0 : h0 + n, :],
                )

            # R6: bh1 segment D, kc-major per bank (all of D has arrived by
            # now) so drains pipeline bank-by-bank: even banks via Act copy
            # + Pool merge, odd banks via direct DVE merge, and the partials
            # ship in hc-halves as soon as their banks are staged.
            for hc in range(NHC):
                for kc in range(*SEGD):
                    mm(1, hc, kc, start=False, stop=(kc == NKC - 1))
                if hc % 2 == 0:
                    nc.scalar.activation(
                        tmps[hc][:], banks[hc][:],
                        mybir.ActivationFunctionType.Copy,
                    )
                    nc.gpsimd.tensor_add(
                        stage[:, 1, :, hc, :], tmps[hc][:], msts[(1, hc)][:]
                    )
                else:
                    nc.vector.tensor_add(
                        stage[:, 1, :, hc, :], banks[hc][:], msts[(1, hc)][:]
                    )
                if hc == 3:
                    drain_dma(1, 0, 4)
                elif hc == 7:
                    drain_dma(1, 4, 4)

            # R7: bh0 segment D; banks freed bank-by-bank by R6's Act
            # copies; drain = DVE add of psum + checkpoint straight into
            # stage, partials shipping in hc-quarters behind the adds.
            for hc in range(NHC):
                for kc in range(*SEGD):
                    mm(0, hc, kc, start=(kc == SEGD[0]), stop=(kc == NKC - 1))
                nc.vector.tensor_add(
                    stage[:, 0, :, hc, :], banks[hc][:], msts[(0, hc)][:]
                )
                if hc % 2 == 1:
                    drain_dma(0, hc - 1, 2)

            nc.gpsimd.collective_compute(
                "ReduceScatter",
                add,
                replica_groups=[list(range(N_CORES))],
                ins=[partials.opt()],
                outs=[rs_out.opt()],
            )

            # --- tail: this core's 128 rows, all transposed layouts.
            # rs_out loads and relus in hc-halves so fc2's first half
            # overlaps the second half's DMA + relu ---
            rsld = hpool.tile([128, NHC, 128], bf16)
            h1t = hpool.tile([128, NHC, 128], bf16)
            rs_ap = rs_out[:].rearrange("(p hc) bw -> p hc bw", p=128)
            half = NHC // 2
            for h0 in (0, half):
                nc.sync.dma_start(
                    out=rsld[:, h0 : h0 + half, :],
                    in_=rs_ap[:, h0 : h0 + half, :],
                )
            for h0 in (0, half):
                # h1 = relu(partial / S1), fused max+scale on DVE (2x bf16)
                nc.vector.tensor_scalar(
                    h1t[:, h0 : h0 + half, :], rsld[:, h0 : h0 + half, :],
                    0.0, 1.0 / S1,
                    mybir.AluOpType.max, mybir.AluOpType.mult,
                )

            # two junk matmuls gated on the RS result absorb the PE's
            # cold-start pstate before the real fc2 work
            for j in range(2):
                nc.tensor.matmul(
                    banks[2][:, 0:128], zl[:], rsld[0:1, 0, :],
                    start=(j == 0), stop=(j == 1), skip_group_check=True,
                )
            # fc2: h2^T = W2^T-blocks @ h1^T chunks; b2 folded in as K=1
            # seed matmuls (zeroing the bank via the first seed's start)
            p_h2 = ppool.tile([128, 512], f32, tag="bank0", name="p_h2")
            for hc2 in range(4):
                nc.tensor.matmul(
                    p_h2[:, hc2 * 128 : (hc2 + 1) * 128],
                    cst[:, B2_OFF + hc2 * 128 : B2_OFF + (hc2 + 1) * 128],
                    cst[:, 0:128],
                    start=(hc2 == 0), stop=False, skip_group_check=True,
                )
            for kc in range(NHC):
                for hc2 in range(4):
                    nc.tensor.matmul(
                        p_h2[:, hc2 * 128 : (hc2 + 1) * 128],
                        wpk_sb[:, (kc * 4 + hc2) * 128 : (kc * 4 + hc2 + 1) * 128],
                        h1t[:, kc, :],
                        start=False,
                        stop=(kc == NHC - 1),
                        skip_group_check=True,
                    )
            # fc3 bias seed issues before the h2 relu so the PE does it
            # while Act runs the relu
            p_outf = ppool.tile([128, 512], f32, tag="bank1", name="p_outf")
            p_out = p_outf[:, 0:C]
            nc.tensor.matmul(
                p_out, cst[:, 0:128], cst[:, 128 : 128 + C],
                start=True, stop=False, skip_group_check=True,
            )
            h2t = hpool.tile([128, 4, 128], bf16)
            nc.scalar.activation(
                h2t[:], p_h2[:], mybir.ActivationFunctionType.Relu
            )
            for hc2 in range(4):
                nc.tensor.matmul(
                    p_out,
                    h2t[:, hc2, :],
                    wpk_sb[:, WOUT_OFF + hc2 * C : WOUT_OFF + (hc2 + 1) * C],
                    start=False,
                    stop=(hc2 == 3),
                    skip_group_check=True,
                )
            o_sb = hpool.tile([128, C], f32)
            nc.scalar.activation(
                o_sb[:], p_out, mybir.ActivationFunctionType.Copy
            )
            nc.scalar.dma_start(out=out_d[:], in_=o_sb[:])

    nc.compile()
    return nc


def _slot_layout(arr2d, cols):
    """[VSH, cols] -> [128, KSUB, cols] with slot s at (s % 128, s // 128)."""
    return np.ascontiguousarray(
        arr2d.reshape(KSUB, 128, cols).transpose(1, 0, 2)
    )


def _shard_inputs(x, W1, b1v, W2, b2v, Wout, boutv):
    x = np.asarray(x).astype(np.int64)
    assert x.shape == (B, S), x.shape
    W1 = np.asarray(W1, dtype=np.float32)
    b1v = np.asarray(b1v, dtype=np.float32)
    w2 = np.asarray(W2, dtype=np.float32)
    wout = np.asarray(Wout, dtype=np.float32)
    b2a = np.asarray(b2v, dtype=np.float32)
    boa = np.asarray(boutv, dtype=np.float32)

    wpk = np.zeros((128, WPK_COLS), dtype=np.float32)
    # w2T blocks: block (kc, hc2) = W2[kc*128:(kc+1)*128, hc2*128:(hc2+1)*128]
    wpk[:, :W2T_COLS] = (
        w2.reshape(NHC, 128, 4, 128).transpose(1, 0, 2, 3).reshape(128, W2T_COLS)
    )
    # woutT blocks: block hc2 = Wout[hc2*128:(hc2+1)*128, :]
    wpk[:, WOUT_OFF:] = (
        wout.reshape(4, 128, C).transpose(1, 0, 2).reshape(128, 4 * C)
    )
    wpk = wpk.astype(BF16)

    consts = np.concatenate(
        [np.ones(128, dtype=np.float32), boa, b2a], axis=0
    ).reshape(1, CST_COLS).astype(BF16)

    shard_of = x.reshape(-1) // SH
    slot_of = x.reshape(-1) % SH
    row_of = np.repeat(np.arange(B, dtype=np.int64), S)

    in_maps = []
    for k in range(N_CORES):
        lo, hi = SH * k, min(SH * (k + 1), V)
        nreal = hi - lo
        wsh = np.zeros((VSH, H1), dtype=np.float32)
        wsh[:nreal] = W1[lo:hi] * np.float32(S1)
        wsh[nreal] = b1v * np.float32(S1 / N_CORES)  # bias row
        wsh8 = _slot_layout(wsh.astype(F8E4), H1)

        sel = shard_of == k
        cnt = np.zeros((VSH, B), dtype=np.float32)
        np.add.at(cnt, (slot_of[sel], row_of[sel]), 1.0)
        cnt[nreal, :] = 1.0  # bias row count
        assert cnt.max() <= 16  # fp8 e4m3 exact-integer range
        cnt8 = _slot_layout(cnt.astype(F8E4), B)

        in_maps.append(
            {"w1s": wsh8, "cnts": cnt8, "wpk": wpk, "consts": consts}
        )
    return in_maps


def modeled_exec_ns():
    """Cost-model (TimelineSim) per-core execution time for the program.

    The axon client in this container has no NTFF profiling hook, so this
    is the best available per-core HW-time estimate.
    """
    global _NC_CACHE
    if _NC_CACHE is None:
        _NC_CACHE = _build_program()
    from concourse.timeline_sim import TimelineSim

    return TimelineSim(_NC_CACHE, trace=False).simulate()


def kernel(x, W1, b1, W2, b2, Wout, bout):
    global _NC_CACHE, LAST_EXEC_NS
    in_maps = _shard_inputs(x, W1, b1, W2, b2, Wout, bout)
    if _NC_CACHE is None:
        _NC_CACHE = _build_program()
    res = run_bass_kernel_spmd(_NC_CACHE, in_maps, list(range(N_CORES)))
    LAST_EXEC_NS = res.exec_time_ns
    out = np.concatenate(
        [np.asarray(res.results[k]["out"]) for k in range(N_CORES)], axis=0
    )
    return out.astype(np.float32)


if __name__ == "__main__":
    rng = np.random.default_rng(0)
    x = rng.integers(0, V, size=(B, S), dtype=np.int64)
    # mirror reference.setup_inputs: uniform(+-1/sqrt(fan_in)), bounded so
    # W1*S1 stays inside fp8 e4m3 range
    W1 = rng.uniform(-1, 1, (V, H1)).astype(np.float32) / np.sqrt(V)
    b1v = rng.uniform(-1, 1, H1).astype(np.float32) / np.sqrt(V)
    W2 = rng.uniform(-1, 1, (H1, H2)).astype(np.float32) / np.sqrt(H1)
    b2v = rng.uniform(-1, 1, H2).astype(np.float32) / np.sqrt(H1)
    Wout = rng.uniform(-1, 1, (H2, C)).astype(np.float32) / np.sqrt(H2)
    bov = rng.uniform(-1, 1, C).astype(np.float32) / np.sqrt(H2)
    got = kernel(x, W1, b1v, W2, b2v, Wout, bov)
    bow = np.zeros((B, V), dtype=np.float32)
    np.add.at(bow, (np.repeat(np.arange(B), S), x.reshape(-1)), 1.0)
    h = np.maximum(bow @ W1 + b1v, 0)
    h = np.maximum(h @ W2 + b2v, 0)
    want = h @ Wout + bov
    err = np.abs(got - want).max() / (np.abs(want).max() + 1e-9)
    print("rel err:", err)
    print("modeled ns:", modeled_exec_ns())


# revision 45
# speedup vs baseline: 1.0221x; 1.0221x over previous
"""BagOfWordsMLP on 8 Trainium2 NeuronCores.

Strategy (vocab-sharded fc1 + single ReduceScatter, transposed layout):
  h1 = relu(bow @ W1 + b1) over a [B=1024, V=50257] histogram. Each core
  owns a 6283-row vocab shard of W1 (pre-scaled by S1, fp8e4m3) and a
  dense fp8 count matrix [6400 slots, 1024 rows] built host-side during
  input sharding. fc1 runs TRANSPOSED on the PE: out = [H1-part,
  batch-free] via DoubleRow fp8 matmuls (w1 stationary, counts moving),
  so partials reach fc2/fc3 already transposed and the tail needs no PE
  transposes or identity matrix. b1 is folded in as an extra vocab slot
  (row = b1*S1/8, count 1).

  PSUM holds 8 of the 16 accumulator groups (8 H1-chunks x 2
  batch-halves). The halves ping-pong through PSUM in the paired round
  order 0A 1A 1B 0B 0C 1C 1D 0D (only 3 occupant changes), with partial
  sums parked in f32 SBUF checkpoints. Occupant changes never reload:
  the incoming half starts fresh (start=True) and its checkpoint is
  merged back by tensor-add at the next park/drain, so transition
  copies are halved and the PE never stalls on a checkpoint round-trip.

  Partials are summed across cores with ONE ReduceScatter of the full
  [B, H1] partial (the cost model charges a 15us constant per
  collective, so one big RS beats two half-RS by 15us). Partials DRAM
  rows are (blk, p, hc) so every DMA descriptor is >= 1KB contiguous,
  and the drain DMAs go out in hc-halves/quarters to overlap the
  final matmuls.
  Each core then computes relu, fc2, fc3 for its own 128 batch rows in
  bf16 with h2 also produced transposed (biases folded in as K=1
  matmul rows).
"""

import os
import sys

import numpy as np

sys.path.insert(0, "/opt/trn_rl_repo")
os.environ.setdefault("JAX_PLATFORMS", "axon,cpu")

import ml_dtypes  # noqa: E402

from concourse import bacc, bass, mybir, tile  # noqa: E402,F401
from concourse.bass_utils import run_bass_kernel_spmd  # noqa: E402

BF16 = ml_dtypes.bfloat16
F8E4 = ml_dtypes.float8_e4m3

N_CORES = 8
B, S = 1024, 512
B_LOC = B // N_CORES  # 128 rows per core
V = 50257
H1, H2, C = 1024, 512, 20

SH = -(-V // N_CORES)  # 6283 vocab rows per shard (last shard 6276)
VSH = 6400  # padded shard slots: 50 k-subtiles, 25 DoubleRow chunks
KSUB = VSH // 128  # 50
NKC = VSH // 256  # 25 DoubleRow chunks
NHC = H1 // 128  # 8 H1 chunks
S1 = 32768.0  # fp8 dequant scale for W1 (max |W1*S1| ~ 146 < e4m3 max)
DR = mybir.MatmulPerfMode.DoubleRow
NJ = 325  # collective-window PE keep-warm matmuls (plateau 300-350)

# kc segments A-D for the PSUM ping-pong (see module docstring)
SEGA, SEGB, SEGC, SEGD = (0, 5), (5, 11), (11, 19), (19, NKC)

# wpk layout: w2T blocks (kc-outer, hc2-inner) | woutT blocks
W2T_COLS = NHC * 4 * 128  # 4096
WOUT_OFF = W2T_COLS
WPK_COLS = WOUT_OFF + 4 * C
# consts layout: ones(128) | bout(C) | b2(512)
B2_OFF = 128 + C
CST_COLS = B2_OFF + H2

LAST_EXEC_NS = None
_NC_CACHE = None


def _build_program():
    nc = bacc.Bacc(
        "TRN2", target_bir_lowering=False, debug=False, num_devices=N_CORES
    )
    f32 = mybir.dt.float32
    bf16 = mybir.dt.bfloat16
    f8e4 = mybir.dt.float8e4
    add = mybir.AluOpType.add

    w1s = nc.declare_dram_parameter("w1s", [128, KSUB, H1], f8e4, isOutput=False)
    cnts = nc.declare_dram_parameter("cnts", [128, KSUB, B], f8e4, isOutput=False)
    wpk = nc.declare_dram_parameter("wpk", [128, WPK_COLS], bf16, isOutput=False)
    consts = nc.declare_dram_parameter("consts", [1, CST_COLS], bf16, isOutput=False)
    out_d = nc.declare_dram_parameter("out", [B_LOC, C], f32, isOutput=True)

    with tile.TileContext(nc) as tc:
        with (
            tc.tile_pool(name="wpool", bufs=1) as wpool,
            tc.tile_pool(name="hpool", bufs=1) as hpool,
            tc.tile_pool(name="ppool", bufs=1, space="PSUM") as ppool,
            tc.tile_pool(name="dram", bufs=1, space="DRAM") as dram,
        ):
            # partials rows = (blk, p, hc); RS scatters blk-major so core k
            # receives batch block k as h1^T [(p hc), bw] with 2KB-contiguous
            # per-partition runs.
            partials = dram.tile([8 * 128 * NHC, 128], bf16, tag="partials",
                                 name="partials")
            rs_out = dram.tile([128 * NHC, 128], bf16, tag="rs_out",
                               name="rs_out")

            # --- PE warmup: junk matmuls on bank0 while the first stream
            # chunk is in flight, so the pstate ramp completes early ---
            zl = wpool.tile([1, 128], bf16)
            zr = wpool.tile([1, 512], bf16)
            nc.vector.memset(zl[:], 0.0)
            nc.vector.memset(zr[:], 0.0)

            # --- stream W1 shard + counts into SBUF (w1 on the SP HWDGE
            # queue, counts on the Pool SWDGE queue) ---
            cst = wpool.tile([1, CST_COLS], bf16)
            nc.scalar.dma_start(out=cst[:], in_=consts[:])
            wpk_sb = wpool.tile([128, WPK_COLS], bf16)
            w1_sb = wpool.tile([128, KSUB, H1], f8e4)
            cnt_sb = wpool.tile([128, KSUB, B], f8e4)
            bounds = [0, 2] + list(range(6, KSUB + 1, 4))
            for i in range(len(bounds) - 1):
                k0, k1 = bounds[i], bounds[i + 1]
                nc.sync.dma_start(out=w1_sb[:, k0:k1, :], in_=w1s[:, k0:k1, :])
                nc.gpsimd.dma_start(out=cnt_sb[:, k0:k1, :], in_=cnts[:, k0:k1, :])
            # fc2/fc3 weights ride the SP queue behind the last w1 chunk
            nc.sync.dma_start(out=wpk_sb[:], in_=wpk[:])

            # stage free layout (bh, blk, hc, bw): per-partition (hc, bw)
            # contiguous within blk -> >=1KB DMA descriptors
            stage = hpool.tile([128, 2, 4, NHC, 128], bf16, tag="stage",
                               name="stage")
            msts = {
                (bh, hc): hpool.tile(
                    [128, 512], f32, tag=f"mst{bh}_{hc}", name=f"mst{bh}_{hc}"
                )
                for bh in range(2)
                for hc in range(NHC)
            }
            tmps = [
                hpool.tile([128, 512], f32, tag=f"tmp{hc}", name=f"tmp{hc}")
                for hc in range(NHC)
            ]

            banks = [
                ppool.tile([128, 512], f32, tag=f"bank{hc}", name=f"bank{hc}")
                for hc in range(NHC)
            ]

            for j in range(4):
                nc.tensor.matmul(
                    banks[0][:], zl[:], zr[:],
                    start=(j == 0), stop=(j == 3), skip_group_check=True,
                )

            def mm(bh, hc, kc, start, stop=False):
                nc.tensor.matmul(
                    banks[hc][:],
                    w1_sb[:, 2 * kc : 2 * kc + 2, hc * 128 : (hc + 1) * 128],
                    cnt_sb[:, 2 * kc : 2 * kc + 2, bh * 512 : (bh + 1) * 512],
                    start=start,
                    stop=stop,
                    perf_mode=DR,
                    skip_group_check=True,
                )

            def kc_inner(bh, seg, fresh):
                a, b = seg
                for kc in range(a, b):
                    for hc in range(NHC):
                        mm(bh, hc, kc, start=(fresh and kc == a),
                           stop=(kc == b - 1))

            def ckpt(bh, hc, eng):
                # park the current psum into this half's checkpoint
                if eng == 0:
                    nc.scalar.activation(
                        msts[(bh, hc)][:], banks[hc][:],
                        mybir.ActivationFunctionType.Copy,
                    )
                else:
                    nc.vector.tensor_copy(msts[(bh, hc)][:], banks[hc][:])

            # R0: bh0 segment A, paced by the stream
            kc_inner(0, SEGA, fresh=True)
            # R1: bh1 segment A; per-bank: park bh0 (alternating Act/DVE),
            # then this bank's matmuls (fresh)
            for hc in range(NHC):
                ckpt(0, hc, 1 - hc % 2)
                for kc in range(*SEGA):
                    mm(1, hc, kc, start=(kc == SEGA[0]))
            # R2: bh1 segment B (continue)
            kc_inner(1, SEGB, fresh=False)
            # R3: bh0 segment B; per-bank: park bh1, matmuls fresh
            for hc in range(NHC):
                ckpt(1, hc, 1 - hc % 2)
                for kc in range(*SEGB):
                    mm(0, hc, kc, start=(kc == SEGB[0]))
            # R4: bh0 segment C (continue)
            kc_inner(0, SEGC, fresh=False)
            # R5: bh1 segment C; per-bank: merge bh0's B+C into its
            # checkpoint (DVE add, psum-capable), matmuls fresh
            for hc in range(NHC):
                nc.vector.tensor_add(
                    msts[(0, hc)][:], banks[hc][:], msts[(0, hc)][:]
                )
                for kc in range(*SEGC):
                    mm(1, hc, kc, start=(kc == SEGC[0]))
            def drain_dma(bh, h0, n, alt=False):
                ap = partials[bh * 4096 : (bh + 1) * 4096, :].rearrange(
                    "(blk p hc) bw -> p blk hc bw", blk=4, p=128
                )
                # bh1 ships on the Act queue, bh0 alternates SP/Act so no
                # quarter queues behind the previous one's issue
                eng = nc.scalar if (bh == 1 or alt) else nc.sync
                eng.dma_start(
                    out=ap[:, :, h0 : h0 + n, :],
                    in_=stage[:, bh, :, h0 : h0 + n, :],
                )

            # R6: bh1 segment D, kc-major per bank (all of D has arrived by
            # now) so drains pipeline bank-by-bank: even banks via Act copy
            # + Pool merge, odd banks via direct DVE merge, and the partials
            # ship in hc-halves as soon as their banks are staged.
            for hc in range(NHC):
                for kc in range(*SEGD):
                    mm(1, hc, kc, start=False, stop=(kc == NKC - 1))
                if hc % 2 == 0:
                    nc.scalar.activation(
                        tmps[hc][:], banks[hc][:],
                        mybir.ActivationFunctionType.Copy,
                    )
                    nc.gpsimd.tensor_add(
                        stage[:, 1, :, hc, :], tmps[hc][:], msts[(1, hc)][:]
                    )
                else:
                    nc.vector.tensor_add(
                        stage[:, 1, :, hc, :], banks[hc][:], msts[(1, hc)][:]
                    )
                if hc == 3:
                    drain_dma(1, 0, 4)
                elif hc == 7:
                    drain_dma(1, 4, 4)

            # R7: bh0 segment D; banks freed bank-by-bank by R6's Act
            # copies; drain = DVE add of psum + checkpoint straight into
            # stage, partials shipping in hc-quarters behind the adds.
            for hc in range(NHC):
                for kc in range(*SEGD):
                    mm(0, hc, kc, start=(kc == SEGD[0]), stop=(kc == NKC - 1))
                nc.vector.tensor_add(
                    stage[:, 0, :, hc, :], banks[hc][:], msts[(0, hc)][:]
                )
                if hc % 2 == 1:
                    drain_dma(0, hc - 1, 2)

            nc.gpsimd.collective_compute(
                "ReduceScatter",
                add,
                replica_groups=[list(range(N_CORES))],
                ins=[partials.opt()],
                outs=[rs_out.opt()],
            )

            # fill the collective window with junk matmuls, gated on the
            # last partials quarter via a tiny tick DMA, sized to end just
            # past the h1 load so fc2/fc3 enter at full PE clock (any PE
            # idle gap would reset the pstate ramp to 1.2GHz)
            tick_sb = hpool.tile([1, 128], bf16)
            nc.gpsimd.dma_start(out=tick_sb[:], in_=partials[6:7, :])
            for j in range(NJ):
                nc.tensor.matmul(
                    banks[3][:, 0:128], zl[:], tick_sb[:],
                    start=(j == 0), stop=(j == NJ - 1),
                    skip_group_check=True,
                )

            # --- tail: this core's 128 rows, all transposed layouts.
            # rs_out loads and relus in hc-halves so fc2's first half
            # overlaps the second half's DMA + relu ---
            rsld = hpool.tile([128, NHC, 128], bf16)
            h1t = hpool.tile([128, NHC, 128], bf16)
            rs_ap = rs_out[:].rearrange("(p hc) bw -> p hc bw", p=128)
            half = NHC // 2
            for h0 in (0, half):
                nc.sync.dma_start(
                    out=rsld[:, h0 : h0 + half, :],
                    in_=rs_ap[:, h0 : h0 + half, :],
                )
            for h0 in (0, half):
                # h1 = relu(partial / S1), fused max+scale on DVE (2x bf16)
                nc.vector.tensor_scalar(
                    h1t[:, h0 : h0 + half, :], rsld[:, h0 : h0 + half, :],
                    0.0, 1.0 / S1,
                    mybir.AluOpType.max, mybir.AluOpType.mult,
                )

            # two junk matmuls gated on the RS result absorb the PE's
            # cold-start pstate before the real fc2 work
            for j in range(2):
                nc.tensor.matmul(
                    banks[2][:, 0:128], zl[:], rsld[0:1, 0, :],
                    start=(j == 0), stop=(j == 1), skip_group_check=True,
                )
            # fc2: h2^T = W2^T-blocks @ h1^T chunks; b2 folded in as K=1
            # seed matmuls (zeroing the bank via the first seed's start)
            p_h2 = ppool.tile([128, 512], f32, tag="bank0", name="p_h2")
            for hc2 in range(4):
                nc.tensor.matmul(
                    p_h2[:, hc2 * 128 : (hc2 + 1) * 128],
                    cst[:, B2_OFF + hc2 * 128 : B2_OFF + (hc2 + 1) * 128],
                    cst[:, 0:128],
                    start=(hc2 == 0), stop=False, skip_group_check=True,
                )
            for kc in range(NHC):
                for hc2 in range(4):
                    nc.tensor.matmul(
                        p_h2[:, hc2 * 128 : (hc2 + 1) * 128],
                        wpk_sb[:, (kc * 4 + hc2) * 128 : (kc * 4 + hc2 + 1) * 128],
                        h1t[:, kc, :],
                        start=False,
                        stop=(kc == NHC - 1),
                        skip_group_check=True,
                    )
            # fc3 bias seed issues before the h2 relu so the PE does it
            # while Act runs the relu
            p_outf = ppool.tile([128, 512], f32, tag="bank1", name="p_outf")
            p_out = p_outf[:, 0:C]
            nc.tensor.matmul(
                p_out, cst[:, 0:128], cst[:, 128 : 128 + C],
                start=True, stop=False, skip_group_check=True,
            )
            h2t = hpool.tile([128, 4, 128], bf16)
            nc.scalar.activation(
                h2t[:], p_h2[:], mybir.ActivationFunctionType.Relu
            )
            for hc2 in range(4):
                nc.tensor.matmul(
                    p_out,
                    h2t[:, hc2, :],
                    wpk_sb[:, WOUT_OFF + hc2 * C : WOUT_OFF + (hc2 + 1) * C],
                    start=False,
                    stop=(hc2 == 3),
                    skip_group_check=True,
                )
            o_sb = hpool.tile([128, C], f32)
            nc.scalar.activation(
                o_sb[:], p_out, mybir.ActivationFunctionType.Copy
            )
            nc.scalar.dma_start(out=out_d[:], in_=o_sb[:])

    nc.compile()
    return nc


def _slot_layout(arr2d, cols):
    """[VSH, cols] -> [128, KSUB, cols] with slot s at (s % 128, s // 128)."""
    return np.ascontiguousarray(
        arr2d.reshape(KSUB, 128, cols).transpose(1, 0, 2)
    )


def _shard_inputs(x, W1, b1v, W2, b2v, Wout, boutv):
    x = np.asarray(x).astype(np.int64)
    assert x.shape == (B, S), x.shape
    W1 = np.asarray(W1, dtype=np.float32)
    b1v = np.asarray(b1v, dtype=np.float32)
    w2 = np.asarray(W2, dtype=np.float32)
    wout = np.asarray(Wout, dtype=np.float32)
    b2a = np.asarray(b2v, dtype=np.float32)
    boa = np.asarray(boutv, dtype=np.float32)

    wpk = np.zeros((128, WPK_COLS), dtype=np.float32)
    # w2T blocks: block (kc, hc2) = W2[kc*128:(kc+1)*128, hc2*128:(hc2+1)*128]
    wpk[:, :W2T_COLS] = (
        w2.reshape(NHC, 128, 4, 128).transpose(1, 0, 2, 3).reshape(128, W2T_COLS)
    )
    # woutT blocks: block hc2 = Wout[hc2*128:(hc2+1)*128, :]
    wpk[:, WOUT_OFF:] = (
        wout.reshape(4, 128, C).transpose(1, 0, 2).reshape(128, 4 * C)
    )
    wpk = wpk.astype(BF16)

    consts = np.concatenate(
        [np.ones(128, dtype=np.float32), boa, b2a], axis=0
    ).reshape(1, CST_COLS).astype(BF16)

    shard_of = x.reshape(-1) // SH
    slot_of = x.reshape(-1) % SH
    row_of = np.repeat(np.arange(B, dtype=np.int64), S)

    in_maps = []
    for k in range(N_CORES):
        lo, hi = SH * k, min(SH * (k + 1), V)
        nreal = hi - lo
        wsh = np.zeros((VSH, H1), dtype=np.float32)
        wsh[:nreal] = W1[lo:hi] * np.float32(S1)
        wsh[nreal] = b1v * np.float32(S1 / N_CORES)  # bias row
        wsh8 = _slot_layout(wsh.astype(F8E4), H1)

        sel = shard_of == k
        cnt = np.zeros((VSH, B), dtype=np.float32)
        np.add.at(cnt, (slot_of[sel], row_of[sel]), 1.0)
        cnt[nreal, :] = 1.0  # bias row count
        assert cnt.max() <= 16  # fp8 e4m3 exact-integer range
        cnt8 = _slot_layout(cnt.astype(F8E4), B)

        in_maps.append(
            {"w1s": wsh8, "cnts": cnt8, "wpk": wpk, "consts": consts}
        )
    return in_maps


def modeled_exec_ns():
    """Cost-model (TimelineSim) per-core execution time for the program.

    The axon client in this container has no NTFF profiling hook, so this
    is the best available per-core HW-time estimate.
    """
    global _NC_CACHE
    if _NC_CACHE is None:
        _NC_CACHE = _build_program()
    from concourse.timeline_sim import TimelineSim

    return TimelineSim(_NC_CACHE, trace=False).simulate()


def kernel(x, W1, b1, W2, b2, Wout, bout):
    global _NC_CACHE, LAST_EXEC_NS
    in_maps = _shard_inputs(x, W1, b1, W2, b2, Wout, bout)
    if _NC_CACHE is None:
        _NC_CACHE = _build_program()
    res = run_bass_kernel_spmd(_NC_CACHE, in_maps, list(range(N_CORES)))
    LAST_EXEC_NS = res.exec_time_ns
    out = np.concatenate(
        [np.asarray(res.results[k]["out"]) for k in range(N_CORES)], axis=0
    )
    return out.astype(np.float32)


if __name__ == "__main__":
    rng = np.random.default_rng(0)
    x = rng.integers(0, V, size=(B, S), dtype=np.int64)
    # mirror reference.setup_inputs: uniform(+-1/sqrt(fan_in)), bounded so
    # W1*S1 stays inside fp8 e4m3 range
    W1 = rng.uniform(-1, 1, (V, H1)).astype(np.float32) / np.sqrt(V)
    b1v = rng.uniform(-1, 1, H1).astype(np.float32) / np.sqrt(V)
    W2 = rng.uniform(-1, 1, (H1, H2)).astype(np.float32) / np.sqrt(H1)
    b2v = rng.uniform(-1, 1, H2).astype(np.float32) / np.sqrt(H1)
    Wout = rng.uniform(-1, 1, (H2, C)).astype(np.float32) / np.sqrt(H2)
    bov = rng.uniform(-1, 1, C).astype(np.float32) / np.sqrt(H2)
    got = kernel(x, W1, b1v, W2, b2v, Wout, bov)
    bow = np.zeros((B, V), dtype=np.float32)
    np.add.at(bow, (np.repeat(np.arange(B), S), x.reshape(-1)), 1.0)
    h = np.maximum(bow @ W1 + b1v, 0)
    h = np.maximum(h @ W2 + b2v, 0)
    want = h @ Wout + bov
    err = np.abs(got - want).max() / (np.abs(want).max() + 1e-9)
    print("rel err:", err)
    print("modeled ns:", modeled_exec_ns())
